# revision 1
# baseline (speedup 1.0000x reference)
"""TRN2 Bass kernel for nn_DebateModel (v1 hybrid).

Device (8 NeuronCores, data-parallel over comments, 8 comments/core):
streams the full token_embed (the memory-dominant input, 201 MB) through
the bidirectional span-encoder input projections
    xp_d = W_ih_d @ x^T   for d in {fwd, bwd}   (fp16 operands, fp32 psum)
which is the bulk of the model's FLOPs and memory traffic.

Host: the sequential LSTM recurrences (latency-bound on TRN2's engines),
span gathers, the per-comment GAT/attention head and the comment
compressor, in fp32 numpy, consuming the device-computed projections.

Self-contained: hardcodes all shapes; no sibling imports.
"""
import sys
import numpy as np

sys.path.insert(0, '/opt/trn_rl_repo')

C, L, FEAT = 64, 1024, 768
H = 80
SPAN = 4 * H            # 320
GATES = 4 * H           # 320 per direction
N_CORES = 8
CPC = C // N_CORES      # comments per core = 8
TOK = CPC * L           # tokens per core = 8192
KCH = FEAT // 128       # 6 contraction chunks
GCH = (2 * GATES) // 128  # 5 gate chunks over both directions (640)
TBLK = 512              # moving-operand token block
NTB = TOK // TBLK       # 16

_compiled = None


def _build():
    import concourse.bass as bass
    import concourse.tile as tile
    from concourse import bacc, mybir
    from contextlib import ExitStack

    f16, f32 = mybir.dt.float16, mybir.dt.float32

    nc = bacc.Bacc("TRN2", target_bir_lowering=False, debug=False,
                   enable_asserts=False, num_devices=N_CORES)

    xt_d = nc.dram_tensor("xt", [KCH, 128, TOK], f16, kind="ExternalInput").ap()
    w_d = nc.dram_tensor("w", [KCH, GCH, 128, 128], f16,
                         kind="ExternalInput").ap()
    xp_d = nc.dram_tensor("xp", [GCH, 128, TOK], f32,
                          kind="ExternalOutput").ap()

    with tile.TileContext(nc) as tc, ExitStack() as ctx:
        wpool = ctx.enter_context(tc.tile_pool(name="w", bufs=1))
        xpool = ctx.enter_context(tc.tile_pool(name="x", bufs=3))
        opool = ctx.enter_context(tc.tile_pool(name="o", bufs=3))
        ppool = ctx.enter_context(tc.tile_pool(name="p", bufs=2, space="PSUM"))

        wt = wpool.tile([128, KCH * GCH * 128], f16)
        wt3 = {}
        for k in range(KCH):
            for g in range(GCH):
                wt3[k, g] = wt[:, bass.ts(k * GCH + g, 128)]
                nc.sync.dma_start(wt3[k, g], w_d[k, g])

        for tb in range(NTB):
            xts = []
            for k in range(KCH):
                xtile = xpool.tile([128, TBLK], f16, tag=f"x{k}")
                nc.sync.dma_start(xtile[:], xt_d[k, :, bass.ts(tb, TBLK)])
                xts.append(xtile)
            for g in range(GCH):
                ps = ppool.tile([128, TBLK], f32, tag="ps")
                for k in range(KCH):
                    nc.tensor.matmul(ps[:], wt3[k, g], xts[k][:],
                                     start=(k == 0), stop=(k == KCH - 1))
                ot = opool.tile([128, TBLK], f32, tag="ot")
                nc.scalar.copy(ot[:], ps[:])
                nc.sync.dma_start(xp_d[g, :, bass.ts(tb, TBLK)], ot[:])
    nc.compile()
    return nc


def _sigmoid(z):
    out = np.empty_like(z)
    np.negative(z, out)
    np.exp(out, out)
    out += 1.0
    np.reciprocal(out, out)
    return out


def _lstm(xp, Whh, b, reverse=False):
    """xp: [L, B, 320] precomputed x @ Wih.T. Returns hidden states
    [L, B, 80] fp32, exact fp32 serial recurrence."""
    Ln, B, _ = xp.shape
    Wt = Whh.T.astype(np.float32)
    h = np.zeros((B, H), np.float32)
    c = np.zeros((B, H), np.float32)
    hs = np.empty((Ln, B, H), np.float32)
    order = range(Ln - 1, -1, -1) if reverse else range(Ln)
    for t in order:
        z = xp[t] + h @ Wt + b
        i, f, g, o = z[:, :H], z[:, H:2*H], z[:, 2*H:3*H], z[:, 3*H:]
        c = _sigmoid(f) * c + _sigmoid(i) * np.tanh(g)
        h = _sigmoid(o) * np.tanh(c)
        hs[t] = h
    return hs


def _attn_pool(feats, vals, mask, W1, b1, W2, b2):
    s = np.maximum(feats @ W1 + b1, 0.0) @ W2 + b2
    s = np.where(mask[:, None], s, -1e9)
    ex = np.exp(s - s.max(0, keepdims=True))
    a = ex / ex.sum(0, keepdims=True)
    a = np.where(mask[:, None], a, 0.0)
    out = (a * vals).sum(0)
    return np.where(mask.any(), out, np.zeros_like(out))


def _gat(h, src, dst, emask, Wm, a_l, a_r, bias):
    An, K = h.shape[0], Wm.shape[0]
    hp = np.stack([h @ Wm[k] for k in range(K)], 1)          # [A, K, D]
    el = (hp * a_l[None]).sum(-1)
    er = (hp * a_r[None]).sum(-1)
    e = el[src] + er[dst]
    e = np.where(e > 0, e, 0.2 * e)
    e = np.where(emask[:, None], e, -1e9)
    m = np.full((An, K), -1e9, np.float32)
    np.maximum.at(m, dst, e)
    ex = np.where(emask[:, None], np.exp(e - m[dst]), 0.0)
    den = np.zeros((An, K), np.float32)
    np.add.at(den, dst, ex)
    alpha = ex / np.maximum(den[dst], 1e-9)
    out = np.zeros((An, K, hp.shape[2]), np.float32)
    np.add.at(out, dst, alpha[:, :, None] * hp[src])
    out = out + bias[None]
    out = np.where(out > 0, out, np.expm1(np.minimum(out, 0.0)))
    return out.reshape(An, -1)


def kernel(**inputs):
    global _compiled
    inp = {k: np.asarray(v) for k, v in inputs.items()}

    # ---- device: input projections over all tokens ----
    token = inp['token_embed'].astype(np.float32)            # [C, L, 768]
    Wih2 = np.concatenate([inp['Wih_f'], inp['Wih_b']], 0)   # [640, 768]
    # pack stationary chunks: w[k, g, p, q] = Wih2[g*128+q, k*128+p]
    wpk = np.ascontiguousarray(
        Wih2.reshape(GCH, 128, KCH, 128).transpose(2, 0, 3, 1)
    ).astype(np.float16)                                     # [6, 5, 128, 128]

    in_maps = []
    for core in range(N_CORES):
        tk = token[core*CPC:(core+1)*CPC]                     # [8, 1024, 768]
        xt = np.ascontiguousarray(
            tk.reshape(TOK, KCH, 128).transpose(1, 2, 0)
        ).astype(np.float16)                                  # [6, 128, 8192]
        in_maps.append(dict(xt=xt, w=wpk))

    if _compiled is None:
        _compiled = _build()
    globals()['_last_in_maps'] = in_maps
    from concourse.bass_utils import run_bass_kernel_spmd
    import time as _time
    _t0 = _time.time()
    res = run_bass_kernel_spmd(_compiled, in_maps,
                               core_ids=list(range(N_CORES)))
    globals()['_last_exec_ns'] = res.exec_time_ns
    globals()['_last_dispatch_s'] = _time.time() - _t0

    xp_all = np.empty((C, L, 2 * GATES), np.float32)
    for core in range(N_CORES):
        xpc = res.results[core]["xp"]                         # [5, 128, 8192]
        xpc = xpc.reshape(2 * GATES, CPC, L).transpose(1, 2, 0)
        xp_all[core*CPC:(core+1)*CPC] = xpc

    # ---- host: recurrences + heads (fp32) ----
    xp_f = np.ascontiguousarray(
        xp_all[:, :, :GATES].transpose(1, 0, 2)) + inp['b_f']  # [L, C, 320]
    xp_b = np.ascontiguousarray(
        xp_all[:, :, GATES:].transpose(1, 0, 2)) + inp['b_b']
    hf = _lstm(xp_f, inp['Whh_f'], 0.0).transpose(1, 0, 2)     # [C, L, 80]
    hb = _lstm(xp_b, inp['Whh_b'], 0.0, reverse=True).transpose(1, 0, 2)

    A = inp['adu_spans'].shape[1]
    W_gat = inp['W_gat'].astype(np.float32)

    def span_rep(c, spans):
        i, j = spans[..., 0], spans[..., 1]
        return np.concatenate([hf[c][j] - hf[c][i - 1], hb[c][i] - hb[c][j + 1],
                               hf[c][i - 1], hb[c][j + 1]], -1)

    rows = []
    for c in range(C):
        cemb = span_rep(c, inp['comment_spans'][c])
        amask = inp['adu_masks'][c]
        adus = span_rep(c, inp['adu_spans'][c]) * amask[:, None]
        isrc, idst = inp['inner_src'][c], inp['inner_dst'][c]
        irel, imask = inp['inner_rel'][c], inp['inner_mask'][c]
        tsrc, tdst = inp['inter_src'][c], inp['inter_dst'][c]
        trel, tmask = inp['inter_rel'][c], inp['inter_mask'][c]
        srcs = [isrc, isrc, tdst, tdst]
        dsts = [idst, idst, tsrc, tsrc]
        masks = [imask & (irel == 0), imask & (irel == 1),
                 tmask & (trel == 0), tmask & (trel == 1)]
        z = np.stack([_gat(adus, srcs[m], dsts[m], masks[m], W_gat[m],
                           inp['a_l'][m], inp['a_r'][m], inp['b_gat'][m])
                      for m in range(4)])                     # [4, A, 768]
        w = np.tanh(z.reshape(4 * A, -1) @ inp['W_sem'] + inp['b_sem'])
        w = (w @ inp['q_sem']).reshape(4, A)
        w = (w * amask[None]).sum(1) / max(amask.sum(), 1)
        beta = np.exp(w - w.max())
        beta /= beta.sum()
        zfin = np.einsum('m,mad->ad', beta, z)
        adu_embeds = zfin @ inp['W_pred'] + inp['b_pred']
        feats = np.concatenate(
            [np.broadcast_to(cemb, (A, SPAN)), adu_embeds], -1)
        att_adu = _attn_pool(feats, adu_embeds, amask & inp['local_masks'][c],
                             inp['W_adu1'], inp['b_adu1'],
                             inp['W_adu2'], inp['b_adu2'])

        def pair(se, de, rel, me, W1, b1, W2, b2):
            onehot = np.stack([rel, 1 - rel], -1).astype(np.float32)
            pe = np.concatenate([adu_embeds[se], adu_embeds[de], onehot], -1)
            fp = np.concatenate(
                [np.broadcast_to(cemb, (pe.shape[0], SPAN)), pe], -1)
            return _attn_pool(fp, pe, me, W1, b1, W2, b2)

        att_inn = pair(isrc, idst, irel, imask, inp['W_inn1'], inp['b_inn1'],
                       inp['W_inn2'], inp['b_inn2'])
        att_int = pair(tdst, tsrc, trel, tmask, inp['W_int1'], inp['b_int1'],
                       inp['W_int2'], inp['b_int2'])
        rows.append(np.concatenate(
            [att_adu, att_inn, att_int, inp['info_scores'][c], cemb]))
    wo_ctx = np.stack(rows).astype(np.float32)                # [64, 1608]

    xpc = (wo_ctx @ inp['Wih_c'].T + inp['b_c'])[:, None, :]  # [64, 1, 800]
    globals()['H'], hs = 200, None
    try:
        hs = _lstm(xpc, inp['Whh_c'], 0.0)[:, 0, :]           # [64, 200]
    finally:
        globals()['H'] = 80
    return np.concatenate([hs, wo_ctx], -1).astype(np.float32)



# revision 19
# speedup vs baseline: 12.2025x; 12.2025x over previous
"""TRN2 Bass kernel for nn_DebateModel (v2: full model on device).

8 NeuronCores, data-parallel over comments (8 comments/core). The entire
per-comment network runs on device: bidirectional span-encoder LSTM
(blocked input projection feeding a For_i recurrence), span gathers
(gpsimd ap_gather), 4-metapath GAT, semantic attention, prediction head
and the three attention pools. Only the tiny 64-step comment-compressor
LSTM runs on host (it is sequential across all 64 comments).

Self-contained: hardcodes all shapes; no sibling imports.
"""
import sys
import numpy as np

sys.path.insert(0, '/opt/trn_rl_repo')

C, L, FEAT = 64, 1024, 768
H = 80                 # LSTM hidden
B = 8                  # comments per core
NCORE = 8
TB = 128               # recurrence block (steps)
NBLK = L // TB         # 8
A = 32                 # ADUs per comment
NS = 33                # spans per comment (comment + ADUs)
E = 48                 # edges per set
D = 256                # GAT head dim
DSEM = 128
SPAN = 320
DWO = 1608
CTX = 200

_compiled = None


# ---------------------------------------------------------------------------
# device kernel
# ---------------------------------------------------------------------------

def build_kernel(debug=False):
    import concourse.bass as bass
    import concourse.tile as tile
    from concourse import bacc, mybir
    from contextlib import ExitStack

    f16, f32 = mybir.dt.float16, mybir.dt.float32
    i16, u8 = mybir.dt.int16, mybir.dt.uint8
    AF = mybir.ActivationFunctionType
    OP = mybir.AluOpType
    ds, ts = bass.ds, bass.ts

    nc = bacc.Bacc("TRN2", target_bir_lowering=False, debug=False,
                   enable_asserts=False, num_devices=NCORE)

    dram = {}

    def din(name, shape, dt=f16):
        dram[name] = nc.dram_tensor(name, shape, dt, kind="ExternalInput").ap()
        return dram[name]

    tok_d = din("tok", [6, 128, B * L])                 # (k6,p128,(b8,c8,t128))
    wih_d = din("wih", [128, 8, 6, H])                  # lhsT (dir,gate),(k)
    whh_d = din("whh", [H, 8, H])                  # lhsT per (dir,gate)
    bg_d = din("bg", [H, 8], f32)                       # bias col per (dir,gate)
    gidx_d = din("gidx", [H, 33], i16)                  # 528 wrapped gather idxs
    amrow_d = din("amrow", [1, 2 * B * NS])             # amask row (2 kinds)
    amcol_d = din("amcol", [A, B])                      # amask col
    invc_d = din("invc", [1, B], f32)                   # 1/max(sum(amask),1)
    mal_d = din("mal", [A, B])                          # amask&lmask col
    mpair_d = din("mpair", [E, 2 * B])                  # raw edge mask col (set,c)
    m12_d = din("m12", [E, B, 12])                      # metapath masks (m4,k3)
    gsrcT_d = din("gsrcT", [A, B, 2, E])                # src one-hot T
    gdstT_d = din("gdstT", [A, B, 2, E])                # dst one-hot T
    gdstE_d = din("gdstE", [E + 1, B, 2, A])            # dst one-hot + ones row
    relT_d = din("relT", [2, B, 2, E])                  # [rel;1-rel] rows
    rel2_d = din("rel2", [E, B, 2, 2])                  # [rel,1-rel] cols
    va_d = din("va", [H, 4, 24])                        # (W_gat@a_lr) chunks
    wgat_d = din("wgat", [H, 4, 12 * D])                # rhs chunks (blk),(mk)
    bgat_d = din("bgat", [1, 12 * D])                   # GAT bias row (mk)
    wsem_d = din("wsem", [128, 6, DSEM])
    bsem_d = din("bsem", [1, DSEM])                     # b_sem - colsum(W_sem)
    qsem_d = din("qsem", [1, DSEM])
    wpred_d = din("wpred", [128, 6, D])
    bpred_d = din("bpred", [1, D])                      # b_pred - colsum(W_pred)
    w1a_d = din("w1a", [H, 4, 48])                      # cemb rows of W*1 (3 pools)
    w1adu_d = din("w1adu", [128, 2, 16])                # adu rows of W_adu1
    w1pe_d = din("w1pe", [128, 2, 4, 16])               # pe rows of W_inn1/int1
    w1rel_d = din("w1rel", [2, 2, 16])                  # rel rows of W_inn1/int1
    b1_d = din("b1", [1, 3, 16])                        # b_adu1,b_inn1,b_int1
    w2_d = din("w2", [1, 3 * 16], f32)                  # W_adu2|W_inn2|W_int2
    ident_d = din("ident", [H, H])
    par_d = din("par", [1, 528], u8)                    # gather parity row

    wo_d = nc.dram_tensor("wo", [B, DWO], f32, kind="ExternalOutput").ap()
    dbg_d = {}
    if debug:
        for nm, shape, dt in [("el", [NS, 24], f16), ("hp", [NS, 12 * D], f16),
                              ("e_sb", [E, 12], f32), ("ex", [E, 12], f16),
                              ("alpha", [E, 12], f32),
                              ("zcT", [128, 6 * 4 * A], f16),
                              ("wall", [A, 4], f16), ("beta", [1, 4], f32),
                              ("ae", [A, D], f16), ("scol", [A, 1], f32),
                              ("rb", [1, 48], f16), ("pe0", [E, 514], f16),
                              ("scol2", [E, 2], f32), ("gfs", [H, 528], f16),
                              ("wsr0", [E + 1, D], f16),
                              ("gbs", [H, 528], f16)]:
            dbg_d[nm] = nc.dram_tensor("dbg_" + nm, shape, dt,
                                       kind="ExternalOutput").ap()

    with tile.TileContext(nc) as tc, ExitStack() as ctx:
        cpool = ctx.enter_context(tc.tile_pool(name="const", bufs=1))
        spool = ctx.enter_context(tc.tile_pool(name="state", bufs=1))
        xpool = ctx.enter_context(tc.tile_pool(name="xp", bufs=2))
        tpool = ctx.enter_context(tc.tile_pool(name="tok", bufs=1))
        wpool = ctx.enter_context(tc.tile_pool(name="work", bufs=3))
        ppool = ctx.enter_context(tc.tile_pool(name="psr", bufs=2, space="PSUM"))
        p1pool = ctx.enter_context(tc.tile_pool(name="ps1", bufs=2, space="PSUM"))

        def cload(ap, shape, dt=f16):
            nm = ap.tensor.name if hasattr(ap, 'tensor') else ap.name
            t = cpool.tile(shape, dt, tag=f"c_{nm}", name=f"c_{nm}")
            nc.sync.dma_start(t[:], ap)
            return t

        wih = cload(wih_d, [128, 8, 6, H])
        whh = cload(whh_d, [H, 8, H])
        bg = cload(bg_d, [H, 8], f32)
        gidx = cload(gidx_d, [H, 33], i16)
        amrow = cload(amrow_d, [1, 2 * B * NS])
        amcol = cload(amcol_d, [A, B])
        invc = cload(invc_d, [1, B], f32)
        mal = cload(mal_d, [A, B])
        mpair = cload(mpair_d, [E, 2 * B])
        m12 = cload(m12_d, [E, B, 12])
        gsrcT = cload(gsrcT_d, [A, B, 2, E])
        gdstT = cload(gdstT_d, [A, B, 2, E])
        gdstE = cload(gdstE_d, [E + 1, B, 2, A])
        relT = cload(relT_d, [2, B, 2, E])
        rel2 = cload(rel2_d, [E, B, 2, 2])
        va = cload(va_d, [H, 4, 24])
        wgat = cload(wgat_d, [H, 4, 12 * D])
        wsem = cload(wsem_d, [128, 6, DSEM])
        bsem = cload(bsem_d, [1, DSEM])
        qsem = cload(qsem_d, [1, DSEM])
        wpred = cload(wpred_d, [128, 6, D])
        bpred = cload(bpred_d, [1, D])
        w1a = cload(w1a_d, [H, 4, 48])
        w1adu = cload(w1adu_d, [128, 2, 16])
        w1pe = cload(w1pe_d, [128, 2, 4, 16])
        w1rel = cload(w1rel_d, [2, 2, 16])
        b1 = cload(b1_d, [1, 3, 16])
        w2 = cload(w2_d, [1, 3 * 16], f32)
        ident = cload(ident_d, [H, H])

        ones_row = cpool.tile([1, 128], f16)
        nc.vector.memset(ones_row[:], 1.0)
        ones_col = cpool.tile([128, 1], f16)
        nc.vector.memset(ones_col[:], 1.0)
        w2bc = cpool.tile([128, 3 * 16], f32)
        nc.gpsimd.partition_broadcast(w2bc[:], w2[:], channels=128)
        amb = cpool.tile([H, 2 * B * NS], f16)
        nc.gpsimd.partition_broadcast(amb[:], amrow[:], channels=H)
        qbc = cpool.tile([A, DSEM], f16)
        nc.gpsimd.partition_broadcast(qbc[:], qsem[:], channels=A)
        # GAT weighted-source tiles; row 48 preloaded with the GAT bias so the
        # dst-scatter matmul (with a ones row in gdstE) adds it for free.
        wsrcs = cpool.tile([E + 1, 12, D], f16)
        nc.sync.dma_start(wsrcs[48:49, :, :], bgat_d)

        # ---- span-encoder LSTM ----
        # h_sf col 1+t holds h_f[t] (col 0 = h[-1] = 0)
        # h_sb col t holds h_b[t]  (col L = h[L] = 0)
        h_sf = spool.tile([H, B, L + 2], f16)
        h_sb = spool.tile([H, B, L + 2], f16)
        nc.vector.memset(h_sf[:, :, 0:1], 0)
        nc.vector.memset(h_sf[:, :, L + 1:L + 2], 0)
        nc.vector.memset(h_sb[:, :, L:L + 2], 0)
        c_st = spool.tile([H, 2, B], f32)
        nc.vector.memset(c_st[:], 0)

        for blk in range(NBLK):
            # phase-1: input projections for fwd block blk / bwd block 7-blk
            xp = [xpool.tile([H, 4, B, TB], f16, tag=f"xp{d}", name=f"xp{d}")
                  for d in range(2)]
            for d in range(2):
                tb = blk if d == 0 else NBLK - 1 - blk
                tks = []
                for k in range(6):
                    tk = tpool.tile([128, B * TB], f16, tag=f"tk{d}{k}")
                    nc.sync.dma_start(tk[:], tok_d[k, :, ts(tb, B * TB)])
                    tks.append(tk)
                for g in range(4):
                    for half in range(2):
                        ps = p1pool.tile([H, 512], f32, tag="p1")
                        for k in range(6):
                            nc.tensor.matmul(
                                ps[:], wih[:, 4 * d + g, k, :],
                                tks[k][:, ts(half, 512)],
                                start=(k == 0), stop=(k == 5))
                        nc.scalar.activation(
                            xp[d][:, g, ts(half, 4), :],
                            ps[:].rearrange("p (c t) -> p c t", c=4),
                            AF.Identity, bias=bg[:, 4 * d + g:4 * d + g + 1])

            base_f = blk * TB
            base_b = L - 1 - blk * TB
            with tc.For_i(0, TB) as i:
                zr = ppool.tile([H, 2, 4, B], f32, tag="zr")
                for d in range(2):
                    hs = h_sf if d == 0 else h_sb
                    col = (base_f + i) if d == 0 else (base_b - i + 1)
                    for g in range(4):
                        nc.tensor.matmul(
                            zr[:, d, g, :], whh[:, 4 * d + g, :],
                            hs[:, :, ds(col, 1)], start=True, stop=True)
                z = wpool.tile([H, 2, 4, B], f32, tag="z")
                nc.vector.tensor_tensor(
                    z[:, 0], xp[0][:, :, :, ds(i, 1)], zr[:, 0], OP.add)
                nc.vector.tensor_tensor(
                    z[:, 1], xp[1][:, :, :, ds(TB - 1 - i, 1)], zr[:, 1], OP.add)
                # gate order (i, f, o, g)
                nc.scalar.activation(z[:, :, 0:3, :], z[:, :, 0:3, :], AF.Sigmoid)
                nc.scalar.activation(z[:, :, 3, :], z[:, :, 3, :], AF.Tanh)
                ig = wpool.tile([H, 2, B], f32, tag="ig")
                nc.vector.tensor_tensor(ig[:], z[:, :, 0, :], z[:, :, 3, :],
                                        OP.mult)
                nc.vector.tensor_tensor(c_st[:], z[:, :, 1, :], c_st[:], OP.mult)
                nc.vector.tensor_tensor(c_st[:], c_st[:], ig[:], OP.add)
                th = wpool.tile([H, 2, B], f32, tag="th")
                nc.scalar.activation(th[:], c_st[:], AF.Tanh)
                nc.vector.tensor_tensor(
                    h_sf[:, :, ds(base_f + i + 1, 1)], z[:, 0, 2, :], th[:, 0],
                    OP.mult)
                nc.vector.tensor_tensor(
                    h_sb[:, :, ds(base_b - i, 1)], z[:, 1, 2, :], th[:, 1],
                    OP.mult)

        # ---- span gathers: gf/gb [H, (kind2, c8, s33)] ----
        par = cload(par_d, [1, 528], u8)
        parb = spool.tile([H, 528], u8)
        nc.gpsimd.partition_broadcast(parb[:], par[:], channels=H)
        gfr = spool.tile([H, 528, 2], f16)
        gbr = spool.tile([H, 528, 2], f16)
        nc.gpsimd.ap_gather(gfr[:], h_sf[:], gidx[:], channels=H,
                            num_elems=B * (L + 2) // 2, d=2, num_idxs=528)
        nc.gpsimd.ap_gather(gbr[:], h_sb[:], gidx[:], channels=H,
                            num_elems=B * (L + 2) // 2, d=2, num_idxs=528)
        gf = spool.tile([H, 528], f16)
        gb = spool.tile([H, 528], f16)
        nc.vector.select(gf[:], parb[:], gfr[:, :, 1], gfr[:, :, 0])
        nc.vector.select(gb[:], parb[:], gbr[:, :, 1], gbr[:, :, 0])
        nc.vector.tensor_tensor(gf[:], gf[:], amb[:], OP.mult)
        nc.vector.tensor_tensor(gb[:], gb[:], amb[:], OP.mult)
        gf3 = gf[:].rearrange("p (k c s) -> p k c s", k=2, c=B)
        gb3 = gb[:].rearrange("p (k c s) -> p k c s", k=2, c=B)

        def dbg(nm, ap):
            if dbg_d:
                nc.sync.dma_start(dbg_d[nm], ap)

        if dbg_d:
            dbg("gfs", gf[:])
            dbg("gbs", gb[:])

        # ---- per-comment heads ----
        for c in range(B):
            # span-rep K-chunks (80 rows each): fj-fi1 | bi-bj1 | fi1 | bj1
            dT = wpool.tile([H, 2, NS], f16, tag="dT")
            nc.vector.tensor_tensor(dT[:, 0], gf3[:, 1, c], gf3[:, 0, c],
                                    OP.subtract)
            nc.vector.tensor_tensor(dT[:, 1], gb3[:, 0, c], gb3[:, 1, c],
                                    OP.subtract)
            spch = [dT[:, 0], dT[:, 1], gf3[:, 0, c], gb3[:, 1, c]]

            # el/er for all (m,k,lr): [NS, 24]
            pel = p1pool.tile([NS, 24], f32, tag="ha")
            for b4 in range(4):
                nc.tensor.matmul(pel[:], spch[b4], va[:, b4, :],
                                 start=(b4 == 0), stop=(b4 == 3))
            el = wpool.tile([NS, 4, 3, 2], f16, tag="el")
            nc.scalar.copy(el[:], pel[:])

            # hp: [NS, (mk12, d256)] f16, two mk at a time
            if dbg_d and c == 0:
                dbg("el", el[:])
            hp = wpool.tile([NS, 12 * D], f16, tag="hp")
            for mk2 in range(6):
                php = p1pool.tile([NS, 2 * D], f32, tag="hb")
                for b4 in range(4):
                    nc.tensor.matmul(php[:], spch[b4],
                                     wgat[:, b4, ts(mk2, 2 * D)],
                                     start=(b4 == 0), stop=(b4 == 3))
                nc.scalar.copy(hp[:, ts(mk2, 2 * D)], php[:])

            # edge logits e[E, (m,k)] = lrelu(el[src] + er[dst]); ex = exp(e)*mask
            if dbg_d and c == 0:
                dbg("hp", hp[:])
            pes = p1pool.tile([E, 12], f32, tag="ha")
            for st in range(2):
                nc.tensor.matmul(pes[:, ts(st, 6)], gsrcT[:, c, st, :],
                                 el[0:32, 2 * st:2 * st + 2, :, 0:1],
                                 start=True, stop=False)
                nc.tensor.matmul(pes[:, ts(st, 6)], gdstT[:, c, st, :],
                                 el[0:32, 2 * st:2 * st + 2, :, 1:2],
                                 start=False, stop=True)
            e_sb = wpool.tile([E, 12], f32, tag="e_sb")
            nc.scalar.copy(e_sb[:], pes[:])
            nc.vector.scalar_tensor_tensor(e_sb[:], pes[:], 0.2, e_sb[:],
                                           OP.mult, OP.max)
            ex = wpool.tile([E, 12], f16, tag="ex")
            nc.scalar.activation(ex[:], e_sb[:], AF.Exp)
            nc.vector.tensor_tensor(ex[:], ex[:], m12[:, c, :], OP.mult)

            # den[A, 12] (clamped) and its reciprocal gathered back to edges
            if dbg_d and c == 0:
                dbg("e_sb", e_sb[:])
                dbg("ex", ex[:])
            pden = p1pool.tile([A, 12], f32, tag="hb")
            for st in range(2):
                nc.tensor.matmul(pden[:, ts(st, 6)], gdstE[0:E, c, st, :],
                                 ex[:, ts(st, 6)], start=True, stop=True)
            rden = wpool.tile([A, 12], f32, tag="rden")
            nc.vector.tensor_scalar_max(rden[:], pden[:], 1e-4)
            nc.vector.reciprocal(rden[:], rden[:])
            rdf = wpool.tile([A, 12], f16, tag="rdf")
            nc.vector.tensor_copy(rdf[:], rden[:])
            prd = p1pool.tile([E, 12], f32, tag="ha")
            for st in range(2):
                nc.tensor.matmul(prd[:, ts(st, 6)], gdstT[:, c, st, :],
                                 rdf[:, ts(st, 6)], start=True, stop=True)
            alpha = wpool.tile([E, 12], f32, tag="alpha")
            nc.vector.tensor_tensor(alpha[:], ex[:], prd[:], OP.mult)

            if dbg_d and c == 0:
                dbg("alpha", alpha[:])
            # GAT aggregate, transposed, with ELU'+1; zcT [128, (k,half), m, a]
            zcT = wpool.tile([128, 6, 4, A], f16, tag="zcT")
            for mk in range(12):
                m, kh = mk // 3, mk % 3
                st = m // 2
                psrc = p1pool.tile([E, D], f32, tag="hb")
                nc.tensor.matmul(psrc[:], gsrcT[:, c, st, :],
                                 hp[0:32, ts(mk, D)], start=True, stop=True)
                nc.vector.tensor_scalar_mul(wsrcs[0:E, mk, :], psrc[:],
                                            alpha[:, mk:mk + 1])
                pz = p1pool.tile([128, 2, A], f32, tag="hb")
                for half in range(2):
                    nc.tensor.matmul(pz[:, half], wsrcs[:, mk, ts(half, 128)],
                                     gdstE[:, c, st, :], start=True, stop=True)
                tmin = wpool.tile([128, 2, A], f32, tag="tmin")
                nc.vector.tensor_scalar_min(tmin[:], pz[:], 0.0)
                nc.scalar.activation(tmin[:], tmin[:], AF.Exp)
                nc.vector.scalar_tensor_tensor(
                    zcT[:, 2 * kh:2 * kh + 2, m, :], pz[:], 0.0, tmin[:],
                    OP.max, OP.add)

            if dbg_d and c == 0:
                dbg("zcT", zcT[:])
                dbg("wsr0", wsrcs[:, 0, :])
            # semantic attention scores
            wall = wpool.tile([A, 4], f16, tag="wall")
            sink = wpool.tile([A, DSEM], f16, tag="sink")
            for m in range(4):
                ps1 = p1pool.tile([A, DSEM], f32, tag="hb")
                for j in range(6):
                    nc.tensor.matmul(ps1[:], zcT[:, j, m, :], wsem[:, j, :],
                                     start=(j == 0), stop=False)
                nc.tensor.matmul(ps1[:], ones_row[:, 0:A], bsem[:],
                                 start=False, stop=True)
                tsb = wpool.tile([A, DSEM], f16, tag="tsb")
                nc.scalar.activation(tsb[:], ps1[:], AF.Tanh)
                nc.vector.scalar_tensor_tensor(
                    sink[:], tsb[:], 0.0, qbc[:], OP.bypass, OP.mult,
                    accum_out=wall[:, m:m + 1])
            pwm = p1pool.tile([1, 4], f32, tag="ha")
            nc.tensor.matmul(pwm[:], amcol[:, c:c + 1], wall[:],
                             start=True, stop=True)
            ex4 = wpool.tile([1, 4], f32, tag="ex4")
            nc.scalar.activation(ex4[:], pwm[:], AF.Exp,
                                 scale=invc[:, c:c + 1])
            sm4 = wpool.tile([1, 2], f32, tag="sm4")
            nc.vector.tensor_reduce(sm4[:, 0:1], ex4[:], mybir.AxisListType.X,
                                    OP.add)
            nc.vector.reciprocal(sm4[:, 1:2], sm4[:, 0:1])
            beta = wpool.tile([1, 4], f32, tag="beta")
            nc.vector.tensor_scalar_mul(beta[:], ex4[:], sm4[:, 1:2])
            if dbg_d and c == 0:
                dbg("wall", wall[:])
                dbg("beta", beta[:])
            bbc = wpool.tile([128, 4], f32, tag="bbc")
            nc.gpsimd.partition_broadcast(bbc[:], beta[:], channels=128)

            # zfinT = sum_m beta_m * zcT_m ; adu embeds
            zfinT = wpool.tile([128, 6, A], f16, tag="zfinT")
            for j in range(6):
                nc.vector.tensor_scalar_mul(zfinT[:, j, :], zcT[:, j, 0, :],
                                            bbc[:, 0:1])
                for m in range(1, 4):
                    nc.vector.scalar_tensor_tensor(
                        zfinT[:, j, :], zcT[:, j, m, :], bbc[:, m:m + 1],
                        zfinT[:, j, :], OP.mult, OP.add)
            pae = p1pool.tile([A, D], f32, tag="hb")
            for j in range(6):
                nc.tensor.matmul(pae[:], zfinT[:, j, :], wpred[:, j, :],
                                 start=(j == 0), stop=False)
            nc.tensor.matmul(pae[:], ones_row[:, 0:A], bpred[:],
                             start=False, stop=True)
            ae = wpool.tile([A, D], f16, tag="ae")
            nc.scalar.copy(ae[:], pae[:])
            if dbg_d and c == 0:
                dbg("ae", ae[:])
            aeT = wpool.tile([128, 2, A], f16, tag="aeT")
            for j in range(2):
                ptr = p1pool.tile([128, A], f16, tag="ha")
                nc.tensor.transpose(ptr[:], ae[:, ts(j, 128)], ident[0:A, 0:A])
                nc.scalar.copy(aeT[:, j, :], ptr[:])

            # cemb contribution to all three pool MLPs: r = cemb @ W*1[:320]
            pr1 = p1pool.tile([1, 48], f32, tag="hb")
            for b4 in range(4):
                nc.tensor.matmul(pr1[:], spch[b4][:, 32:33], w1a[:, b4, :],
                                 start=(b4 == 0), stop=(b4 == 3))
            rb = wpool.tile([1, 3, 16], f16, tag="rb")
            nc.vector.tensor_tensor(rb[:], pr1[:], b1[:], OP.add)

            out_row = wpool.tile([1, DWO], f32, tag="out_row")
            nc.vector.memset(out_row[:, 1284:1288], 0)

            # --- ADU attention pool ---
            ps1a = p1pool.tile([A, 16], f32, tag="ha")
            for j in range(2):
                nc.tensor.matmul(ps1a[:], aeT[:, j, :], w1adu[:, j, :],
                                 start=(j == 0), stop=False)
            nc.tensor.matmul(ps1a[:], ones_row[:, 0:A], rb[:, 0, :],
                             start=False, stop=True)
            scol = wpool.tile([A, 1], f32, tag="scol")
            sink16 = wpool.tile([A, 16], f16, tag="sink16")
            nc.vector.scalar_tensor_tensor(sink16[:], ps1a[:], 0.0,
                                           w2bc[0:A, 0:16], OP.max, OP.mult,
                                           accum_out=scol[:])
            if dbg_d and c == 0:
                dbg("scol", scol[:])
                dbg("rb", rb[:])
            exm = wpool.tile([A, 1], f16, tag="exm")
            nc.scalar.activation(exm[:], scol[:], AF.Exp)
            nc.vector.tensor_tensor(exm[:], exm[:], mal[:, c:c + 1], OP.mult)
            pd1 = p1pool.tile([1, 1], f32, tag="hb")
            nc.tensor.matmul(pd1[:], exm[:], ones_col[0:A, :],
                             start=True, stop=True)
            rd1 = wpool.tile([1, 1], f32, tag="rd1")
            nc.vector.tensor_scalar_max(rd1[:], pd1[:], 1e-30)
            nc.vector.reciprocal(rd1[:], rd1[:])
            ppl = p1pool.tile([1, D], f32, tag="ha")
            nc.tensor.matmul(ppl[:], exm[:], ae[:], start=True, stop=True)
            nc.vector.tensor_scalar_mul(out_row[:, 0:256], ppl[:], rd1[:])

            # --- pair attention pools (inner st=0, inter st=1) ---
            for st in range(2):
                base = 256 + 514 * st
                pe = wpool.tile([E, 514], f16, tag="pe")
                pse = p1pool.tile([E, D], f32, tag="hb")
                nc.tensor.matmul(pse[:], gsrcT[:, c, st, :], ae[:],
                                 start=True, stop=True)
                nc.scalar.copy(pe[:, 0:256], pse[:])
                psd = p1pool.tile([E, D], f32, tag="ha")
                nc.tensor.matmul(psd[:], gdstT[:, c, st, :], ae[:],
                                 start=True, stop=True)
                nc.scalar.copy(pe[:, 256:512], psd[:])
                nc.scalar.copy(pe[:, 512:514], rel2[:, c, st, :])
                if dbg_d and c == 0 and st == 0:
                    dbg("pe0", pe[:])
                peT = wpool.tile([128, 4, E], f16, tag="peT")
                for j in range(4):
                    gT = gsrcT if j < 2 else gdstT
                    ptp = p1pool.tile([128, E], f32, tag="hb")
                    nc.tensor.matmul(ptp[:], ae[:, ts(j % 2, 128)],
                                     gT[:, c, st, :], start=True, stop=True)
                    nc.scalar.copy(peT[:, j, :], ptp[:])
                ps1p = p1pool.tile([E, 16], f32, tag="ha")
                for j in range(4):
                    nc.tensor.matmul(ps1p[:], peT[:, j, :], w1pe[:, st, j, :],
                                     start=(j == 0), stop=False)
                nc.tensor.matmul(ps1p[:], relT[:, c, st, :], w1rel[:, st, :],
                                 start=False, stop=False)
                nc.tensor.matmul(ps1p[:], ones_row[:, 0:E], rb[:, 1 + st, :],
                                 start=False, stop=True)
                scol2 = wpool.tile([E, 1], f32, tag="scol2")
                sink2 = wpool.tile([E, 16], f16, tag="sink2")
                nc.vector.scalar_tensor_tensor(
                    sink2[:], ps1p[:], 0.0, w2bc[0:E, ts(1 + st, 16)],
                    OP.max, OP.mult, accum_out=scol2[:])
                if dbg_d and c == 0:
                    nc.sync.dma_start(dbg_d["scol2"][:, st:st + 1], scol2[:])
                exm2 = wpool.tile([E, 1], f16, tag="exm2")
                nc.scalar.activation(exm2[:], scol2[:], AF.Exp)
                nc.vector.tensor_tensor(exm2[:], exm2[:],
                                        mpair[:, st * B + c:st * B + c + 1],
                                        OP.mult)
                pd2 = p1pool.tile([1, 1], f32, tag="hb")
                nc.tensor.matmul(pd2[:], exm2[:], ones_col[0:E, :],
                                 start=True, stop=True)
                rd2 = wpool.tile([1, 1], f32, tag="rd2")
                nc.vector.tensor_scalar_max(rd2[:], pd2[:], 1e-30)
                nc.vector.reciprocal(rd2[:], rd2[:])
                pp1 = p1pool.tile([1, 512], f32, tag="ha")
                nc.tensor.matmul(pp1[:], exm2[:], pe[:, 0:512],
                                 start=True, stop=True)
                pp2 = p1pool.tile([1, 2], f32, tag="hb")
                nc.tensor.matmul(pp2[:], exm2[:], pe[:, 512:514],
                                 start=True, stop=True)
                nc.vector.tensor_scalar_mul(out_row[:, base:base + 512],
                                            pp1[:], rd2[:])
                nc.vector.tensor_scalar_mul(out_row[:, base + 512:base + 514],
                                            pp2[:], rd2[:])

            # cemb -> out_row[1288:1608]
            pcb = p1pool.tile([1, 4, H], f32, tag="ha")
            for b4 in range(4):
                nc.tensor.matmul(pcb[:, b4, :], spch[b4][:, 32:33], ident[:],
                                 start=True, stop=True)
            nc.scalar.copy(out_row[:, 1288:1608], pcb[:])

            nc.sync.dma_start(wo_d[c:c + 1, :], out_row[:])

    nc.compile()
    return nc


# ---------------------------------------------------------------------------
# host-side packing
# ---------------------------------------------------------------------------

def _wrap16(flat, channels):
    """Wrap a flat index list into ap_gather's [channels, n//16] layout."""
    n = len(flat)
    out = np.zeros((channels, n // 16), np.int16)
    j = np.arange(n)
    for g in range(channels // 16):
        out[g * 16 + j % 16, j // 16] = flat
    return out


def pack_weights(inp):
    """Pack shared weights (identical for every core)."""
    f16 = np.float16
    perm = np.r_[0:80, 80:160, 240:320, 160:240]     # torch ifgo -> (i,f,o,g)

    w = {}
    wih2 = np.stack([inp['Wih_f'][perm], inp['Wih_b'][perm]])   # [2,320,768]
    # lhsT chunks [128, (dir,gate), k, 80]: wih[p, dg, k, m] = wih2[d, g*80+m, k*128+p]
    w['wih'] = np.ascontiguousarray(
        wih2.reshape(2, 4, H, 6, 128).transpose(4, 0, 1, 3, 2)
    ).reshape(128, 8, 6, H).astype(f16)
    whh2 = np.stack([inp['Whh_f'][perm], inp['Whh_b'][perm]])   # [2,320,80]
    w['whh'] = np.ascontiguousarray(
        whh2.reshape(2, 4, H, H).transpose(3, 0, 1, 2)
    ).reshape(H, 8, H).astype(f16)
    bg2 = np.stack([inp['b_f'][perm], inp['b_b'][perm]])        # [2,320]
    w['bg'] = np.ascontiguousarray(
        bg2.reshape(2, 4, H).transpose(2, 0, 1)).reshape(H, 8).astype(np.float32)

    wg = inp['W_gat'].astype(np.float32)                        # [4,3,320,256]
    va = np.einsum('mkdo,mko->mkd', wg, inp['a_l'])             # [4,3,320]
    vb = np.einsum('mkdo,mko->mkd', wg, inp['a_r'])
    vab = np.stack([va, vb], -1).reshape(4, 3, 4, H, 2)         # m,k,blk,80,lr
    w['va'] = np.ascontiguousarray(
        vab.transpose(3, 2, 0, 1, 4)).reshape(H, 4, 24).astype(f16)
    # wgat rhs chunks [80, blk, (mk, 256)]
    w['wgat'] = np.ascontiguousarray(
        wg.reshape(4, 3, 4, H, D).transpose(3, 2, 0, 1, 4)
    ).reshape(H, 4, 12 * D).astype(f16)
    w['bgat'] = inp['b_gat'].reshape(1, 12 * D).astype(f16)

    ws = inp['W_sem'].astype(np.float32)                        # [768,128]
    w['wsem'] = np.ascontiguousarray(
        ws.reshape(6, 128, DSEM).transpose(1, 0, 2)).astype(f16)
    w['bsem'] = (inp['b_sem'] - ws.sum(0)).reshape(1, DSEM).astype(f16)
    w['qsem'] = inp['q_sem'].reshape(1, DSEM).astype(f16)
    wp = inp['W_pred'].astype(np.float32)                       # [768,256]
    w['wpred'] = np.ascontiguousarray(
        wp.reshape(6, 128, D).transpose(1, 0, 2)).astype(f16)
    w['bpred'] = (inp['b_pred'] - wp.sum(0)).reshape(1, D).astype(f16)

    w1s = [inp['W_adu1'].astype(np.float32), inp['W_inn1'].astype(np.float32),
           inp['W_int1'].astype(np.float32)]
    w1a = np.stack([wm[:SPAN].reshape(4, H, 16) for wm in w1s], 2)  # blk,80,pool,16
    w['w1a'] = np.ascontiguousarray(w1a.transpose(1, 0, 2, 3)).reshape(
        H, 4, 48).astype(f16)
    w['w1adu'] = np.ascontiguousarray(
        w1s[0][SPAN:SPAN + D].reshape(2, 128, 16).transpose(1, 0, 2)
    ).astype(f16)
    w1pe = np.stack([w1s[1][SPAN:SPAN + 512], w1s[2][SPAN:SPAN + 512]])
    w['w1pe'] = np.ascontiguousarray(
        w1pe.reshape(2, 4, 128, 16).transpose(2, 0, 1, 3)).astype(f16)
    w['w1rel'] = np.stack([w1s[1][832:834], w1s[2][832:834]]).transpose(
        1, 0, 2).astype(f16)
    w['b1'] = np.stack([inp['b_adu1'], inp['b_inn1'], inp['b_int1']]).reshape(
        1, 3, 16).astype(f16)
    w['w2'] = np.stack([inp['W_adu2'][:, 0], inp['W_inn2'][:, 0],
                        inp['W_int2'][:, 0]]).reshape(1, 48).astype(np.float32)
    w['ident'] = np.eye(H, dtype=f16)
    return w


def pack_core(inp, core, wshared):
    """Per-core inputs (tokens, graph one-hots, masks, gather indices)."""
    f16 = np.float16
    sl = slice(core * B, (core + 1) * B)
    m = dict(wshared)

    tok = inp['token_embed'][sl].astype(f16)                    # [8,1024,768]
    m['tok'] = np.ascontiguousarray(
        tok.reshape(B, NBLK, TB, 6, 128).transpose(3, 4, 1, 0, 2)
    ).reshape(6, 128, B * L)

    cs = inp['comment_spans'][sl].astype(np.int64)              # [8,2]
    asp = inp['adu_spans'][sl].astype(np.int64)                 # [8,32,2]
    i_all = np.concatenate([asp[:, :, 0], cs[:, None, 0]], 1)   # [8,33]
    j_all = np.concatenate([asp[:, :, 1], cs[:, None, 1]], 1)
    coff = np.arange(B)[:, None] * (L + 2)
    flat = np.concatenate([(coff + i_all).ravel(),
                           (coff + j_all + 1).ravel()]).astype(np.int64)
    m['gidx'] = _wrap16((flat >> 1).astype(np.int16), H)
    m['par'] = (flat & 1).reshape(1, 528).astype(np.uint8)

    am = inp['adu_masks'][sl].astype(np.float32)                # [8,32]
    am_ext = np.concatenate([am, np.ones((B, 1), np.float32)], 1)  # [8,33]
    m['amrow'] = np.tile(am_ext.ravel(), 2).reshape(1, 2 * B * NS).astype(f16)
    m['amcol'] = am.T.astype(f16)
    m['invc'] = (1.0 / np.maximum(am.sum(1), 1.0)).reshape(1, B).astype(
        np.float32)
    m['mal'] = (am * inp['local_masks'][sl]).T.astype(f16)

    im = inp['inner_mask'][sl].astype(np.float32)               # [8,48]
    tm = inp['inter_mask'][sl].astype(np.float32)
    m['mpair'] = np.concatenate([im, tm], 0).T.astype(f16)      # [48, 2*8]
    irel = inp['inner_rel'][sl].astype(np.int64)
    trel = inp['inter_rel'][sl].astype(np.int64)
    m4 = np.stack([im * (irel == 0), im * (irel == 1),
                   tm * (trel == 0), tm * (trel == 1)], 1)      # [8,4,48]
    m['m12'] = np.ascontiguousarray(
        np.repeat(m4[:, :, None, :], 3, 2).transpose(3, 0, 1, 2)
    ).reshape(E, B, 12).astype(f16)

    eye = np.eye(A, dtype=np.float32)
    src = np.stack([inp['inner_src'][sl], inp['inter_dst'][sl]], 1)  # [8,2,48]
    dst = np.stack([inp['inner_dst'][sl], inp['inter_src'][sl]], 1)
    gsrc = eye[src.astype(np.int64)]                            # [8,2,48,32]
    gdst = eye[dst.astype(np.int64)]
    m['gsrcT'] = np.ascontiguousarray(
        gsrc.transpose(3, 0, 1, 2)).astype(f16)                 # [32,8,2,48]
    m['gdstT'] = np.ascontiguousarray(gdst.transpose(3, 0, 1, 2)).astype(f16)
    gde = np.concatenate([gdst, np.ones((B, 2, 1, A), np.float32)], 2)
    m['gdstE'] = np.ascontiguousarray(
        gde.transpose(2, 0, 1, 3)).astype(f16)                  # [49,8,2,32]
    rel = np.stack([irel, trel], 1).astype(np.float32)          # [8,2,48]
    r2 = np.stack([rel, 1.0 - rel], -1)                         # [8,2,48,2]
    m['relT'] = np.ascontiguousarray(
        r2.transpose(3, 0, 1, 2)).astype(f16)                   # [2,8,2,48]
    m['rel2'] = np.ascontiguousarray(
        r2.transpose(2, 0, 1, 3)).astype(f16)                   # [48,8,2,2]
    return m


# ---------------------------------------------------------------------------
# host-side epilogue (comment-compressor LSTM)
# ---------------------------------------------------------------------------

def compressor(wo, inp):
    Wt = inp['Whh_c'].T.astype(np.float32)
    xp = wo @ inp['Wih_c'].T.astype(np.float32) + inp['b_c']    # [64, 800]
    h = np.zeros(CTX, np.float32)
    c = np.zeros(CTX, np.float32)
    hs = np.empty((C, CTX), np.float32)

    def sig(v):
        return 1.0 / (1.0 + np.exp(-v))

    for t in range(C):
        z = xp[t] + h @ Wt
        i, f, g, o = (z[:CTX], z[CTX:2 * CTX], z[2 * CTX:3 * CTX], z[3 * CTX:])
        c = sig(f) * c + sig(i) * np.tanh(g)
        h = sig(o) * np.tanh(c)
        hs[t] = h
    return hs


# ---------------------------------------------------------------------------
# entry point
# ---------------------------------------------------------------------------

def kernel(**inputs):
    global _compiled
    inp = {k: np.asarray(v) for k, v in inputs.items()}

    wshared = pack_weights(inp)
    in_maps = [pack_core(inp, core, wshared) for core in range(NCORE)]

    if _compiled is None:
        _compiled = build_kernel()
    globals()['_last_in_maps'] = in_maps

    from concourse.bass_utils import run_bass_kernel_spmd
    import time as _time
    _t0 = _time.time()
    res = run_bass_kernel_spmd(_compiled, in_maps,
                               core_ids=list(range(NCORE)))
    globals()['_last_exec_ns'] = res.exec_time_ns
    globals()['_last_dispatch_s'] = _time.time() - _t0

    wo = np.concatenate([res.results[core]["wo"] for core in range(NCORE)], 0)
    wo[:, 1284:1288] = inp['info_scores'].astype(np.float32)

    ctx = compressor(wo, inp)
    return np.concatenate([ctx, wo], -1).astype(np.float32)


# revision 21
# speedup vs baseline: 15.1937x; 1.2451x over previous
"""TRN2 Bass kernel for nn_DebateModel (v2: full model on device).

8 NeuronCores, data-parallel over comments (8 comments/core). The entire
per-comment network runs on device: bidirectional span-encoder LSTM
(blocked input projection feeding a For_i recurrence), span gathers
(gpsimd ap_gather), 4-metapath GAT, semantic attention, prediction head
and the three attention pools. Only the tiny 64-step comment-compressor
LSTM runs on host (it is sequential across all 64 comments).

Self-contained: hardcodes all shapes; no sibling imports.
"""
import sys
import numpy as np

sys.path.insert(0, '/opt/trn_rl_repo')

C, L, FEAT = 64, 1024, 768
H = 80                 # LSTM hidden
B = 8                  # comments per core
NCORE = 8
TB = 128               # recurrence block (steps)
NBLK = L // TB         # 8
A = 32                 # ADUs per comment
NS = 33                # spans per comment (comment + ADUs)
E = 48                 # edges per set
D = 256                # GAT head dim
DSEM = 128
SPAN = 320
DWO = 1608
CTX = 200

_compiled = None


# ---------------------------------------------------------------------------
# device kernel
# ---------------------------------------------------------------------------

def build_kernel(debug=False):
    import concourse.bass as bass
    import concourse.tile as tile
    from concourse import bacc, mybir
    from contextlib import ExitStack

    f16, f32 = mybir.dt.float16, mybir.dt.float32
    i16, u8 = mybir.dt.int16, mybir.dt.uint8
    f8 = mybir.dt.float8e4
    AF = mybir.ActivationFunctionType
    OP = mybir.AluOpType
    ds, ts = bass.ds, bass.ts

    nc = bacc.Bacc("TRN2", target_bir_lowering=False, debug=False,
                   enable_asserts=False, num_devices=NCORE)

    dram = {}

    def din(name, shape, dt=f16):
        dram[name] = nc.dram_tensor(name, shape, dt, kind="ExternalInput").ap()
        return dram[name]

    tok_d = din("tok", [6, 128, B * L], mybir.dt.int8)             # (k6,p128,(b8,c8,t128))
    wih_d = din("wih", [128, 8, 6, H])                  # lhsT (dir,gate),(k)
    whh_d = din("whh", [H, 8, H])                  # lhsT per (dir,gate)
    bg_d = din("bg", [H, 8], f32)                       # bias col per (dir,gate)
    gidx_d = din("gidx", [H, 33], i16)                  # 528 wrapped gather idxs
    amrow_d = din("amrow", [1, 2 * B * NS])             # amask row (2 kinds)
    amcol_d = din("amcol", [A, B])                      # amask col
    invc_d = din("invc", [1, B], f32)                   # 1/max(sum(amask),1)
    mal_d = din("mal", [A, B])                          # amask&lmask col
    mpair_d = din("mpair", [E, 2 * B])                  # raw edge mask col (set,c)
    m12_d = din("m12", [E, B, 12])                      # metapath masks (m4,k3)
    gsrcT_d = din("gsrcT", [A, B, 2, E])                # src one-hot T
    gdstT_d = din("gdstT", [A, B, 2, E])                # dst one-hot T
    gdstE_d = din("gdstE", [E + 1, B, 2, A])            # dst one-hot + ones row
    relT_d = din("relT", [2, B, 2, E])                  # [rel;1-rel] rows
    rel2_d = din("rel2", [E, B, 2, 2])                  # [rel,1-rel] cols
    va_d = din("va", [H, 4, 24])                        # (W_gat@a_lr) chunks
    wgat_d = din("wgat", [H, 4, 12 * D])                # rhs chunks (blk),(mk)
    bgat_d = din("bgat", [1, 12 * D])                   # GAT bias row (mk)
    wsem_d = din("wsem", [128, 6, DSEM])
    bsem_d = din("bsem", [1, DSEM])                     # b_sem - colsum(W_sem)
    qsem_d = din("qsem", [1, DSEM])
    wpred_d = din("wpred", [128, 6, D])
    bpred_d = din("bpred", [1, D])                      # b_pred - colsum(W_pred)
    w1a_d = din("w1a", [H, 4, 48])                      # cemb rows of W*1 (3 pools)
    w1adu_d = din("w1adu", [128, 2, 16])                # adu rows of W_adu1
    w1pe_d = din("w1pe", [128, 2, 4, 16])               # pe rows of W_inn1/int1
    w1rel_d = din("w1rel", [2, 2, 16])                  # rel rows of W_inn1/int1
    b1_d = din("b1", [1, 3, 16])                        # b_adu1,b_inn1,b_int1
    w2_d = din("w2", [1, 3 * 16], f32)                  # W_adu2|W_inn2|W_int2
    ident_d = din("ident", [H, H])
    par_d = din("par", [1, 528], u8)                    # gather parity row

    wo_d = nc.dram_tensor("wo", [B, DWO], f32, kind="ExternalOutput").ap()
    dbg_d = {}
    if debug:
        for nm, shape, dt in [("el", [NS, 24], f16), ("hp", [NS, 12 * D], f16),
                              ("e_sb", [E, 12], f32), ("ex", [E, 12], f16),
                              ("alpha", [E, 12], f32),
                              ("zcT", [128, 6 * 4 * A], f16),
                              ("wall", [A, 4], f16), ("beta", [1, 4], f32),
                              ("ae", [A, D], f16), ("scol", [A, 1], f32),
                              ("rb", [1, 48], f16), ("pe0", [E, 514], f16),
                              ("scol2", [E, 2], f32), ("gfs", [H, 528], f16),
                              ("wsr0", [E + 1, D], f16),
                              ("gbs", [H, 528], f16)]:
            dbg_d[nm] = nc.dram_tensor("dbg_" + nm, shape, dt,
                                       kind="ExternalOutput").ap()

    with tile.TileContext(nc) as tc, ExitStack() as ctx:
        cpool = ctx.enter_context(tc.tile_pool(name="const", bufs=1))
        spool = ctx.enter_context(tc.tile_pool(name="state", bufs=1))
        xpool = ctx.enter_context(tc.tile_pool(name="xp", bufs=2))
        tpool = ctx.enter_context(tc.tile_pool(name="tok", bufs=1))
        wpool = ctx.enter_context(tc.tile_pool(name="work", bufs=3))
        ppool = ctx.enter_context(tc.tile_pool(name="psr", bufs=2, space="PSUM"))
        p1pool = ctx.enter_context(tc.tile_pool(name="ps1", bufs=2, space="PSUM"))

        def cload(ap, shape, dt=f16):
            nm = ap.tensor.name if hasattr(ap, 'tensor') else ap.name
            t = cpool.tile(shape, dt, tag=f"c_{nm}", name=f"c_{nm}")
            nc.sync.dma_start(t[:], ap)
            return t

        wih = cload(wih_d, [128, 8, 6, H])
        whh = cload(whh_d, [H, 8, H])
        bg = cload(bg_d, [H, 8], f32)
        gidx = cload(gidx_d, [H, 33], i16)
        amrow = cload(amrow_d, [1, 2 * B * NS])
        amcol = cload(amcol_d, [A, B])
        invc = cload(invc_d, [1, B], f32)
        mal = cload(mal_d, [A, B])
        mpair = cload(mpair_d, [E, 2 * B])
        m12 = cload(m12_d, [E, B, 12])
        gsrcT = cload(gsrcT_d, [A, B, 2, E])
        gdstT = cload(gdstT_d, [A, B, 2, E])
        gdstE = cload(gdstE_d, [E + 1, B, 2, A])
        relT = cload(relT_d, [2, B, 2, E])
        rel2 = cload(rel2_d, [E, B, 2, 2])
        va = cload(va_d, [H, 4, 24])
        wgat = cload(wgat_d, [H, 4, 12 * D])
        wsem = cload(wsem_d, [128, 6, DSEM])
        bsem = cload(bsem_d, [1, DSEM])
        qsem = cload(qsem_d, [1, DSEM])
        wpred = cload(wpred_d, [128, 6, D])
        bpred = cload(bpred_d, [1, D])
        w1a = cload(w1a_d, [H, 4, 48])
        w1adu = cload(w1adu_d, [128, 2, 16])
        w1pe = cload(w1pe_d, [128, 2, 4, 16])
        w1rel = cload(w1rel_d, [2, 2, 16])
        b1 = cload(b1_d, [1, 3, 16])
        w2 = cload(w2_d, [1, 3 * 16], f32)
        ident = cload(ident_d, [H, H])

        ones_row = cpool.tile([1, 128], f16)
        nc.vector.memset(ones_row[:], 1.0)
        ones_col = cpool.tile([128, 1], f16)
        nc.vector.memset(ones_col[:], 1.0)
        w2bc = cpool.tile([128, 3 * 16], f32)
        nc.gpsimd.partition_broadcast(w2bc[:], w2[:], channels=128)
        amb = cpool.tile([H, 2 * B * NS], f16)
        nc.gpsimd.partition_broadcast(amb[:], amrow[:], channels=H)
        qbc = cpool.tile([A, DSEM], f16)
        nc.gpsimd.partition_broadcast(qbc[:], qsem[:], channels=A)

        # ---- span-encoder LSTM ----
        # h_sf col 1+t holds h_f[t] (col 0 = h[-1] = 0)
        # h_sb col t holds h_b[t]  (col L = h[L] = 0)
        h_sf = spool.tile([H, B, L + 2], f16)
        h_sb = spool.tile([H, B, L + 2], f16)
        nc.vector.memset(h_sf[:, :, 0:1], 0)
        nc.vector.memset(h_sf[:, :, L + 1:L + 2], 0)
        nc.vector.memset(h_sb[:, :, L:L + 2], 0)
        c_st = spool.tile([H, 2, B], f32)
        nc.vector.memset(c_st[:], 0)

        for blk in range(NBLK):
            # phase-1: input projections for fwd block blk / bwd block 7-blk
            xp = [xpool.tile([H, 4, B, TB], f16, tag=f"xp{d}", name=f"xp{d}")
                  for d in range(2)]
            for d in range(2):
                tb = blk if d == 0 else NBLK - 1 - blk
                tks = []
                for k in range(6):
                    tk8 = tpool.tile([128, B * TB], mybir.dt.int8,
                                     tag=f"tk{d}{k}", name=f"tk8_{d}{k}")
                    nc.sync.dma_start(tk8[:], tok_d[k, :, ts(tb, B * TB)])
                    tk = tpool.tile([128, B * TB], f16, tag=f"stg{k}",
                                    name=f"stg{k}")
                    nc.vector.tensor_copy(tk[:], tk8[:])
                    tks.append(tk)
                for g in range(4):
                    for half in range(2):
                        ps = p1pool.tile([H, 512], f32, tag="p1")
                        for k in range(6):
                            nc.tensor.matmul(
                                ps[:], wih[:, 4 * d + g, k, :],
                                tks[k][:, ts(half, 512)],
                                start=(k == 0), stop=(k == 5))
                        nc.scalar.activation(
                            xp[d][:, g, ts(half, 4), :],
                            ps[:].rearrange("p (c t) -> p c t", c=4),
                            AF.Identity, bias=bg[:, 4 * d + g:4 * d + g + 1])

            base_f = blk * TB
            base_b = L - 1 - blk * TB
            with tc.For_i(0, TB) as i:
                zr = ppool.tile([H, 2, 4, B], f32, tag="zr")
                for d in range(2):
                    hs = h_sf if d == 0 else h_sb
                    col = (base_f + i) if d == 0 else (base_b - i + 1)
                    for g in range(4):
                        nc.tensor.matmul(
                            zr[:, d, g, :], whh[:, 4 * d + g, :],
                            hs[:, :, ds(col, 1)], start=True, stop=True)
                z = wpool.tile([H, 2, 4, B], f32, tag="z")
                nc.vector.tensor_tensor(
                    z[:, 0], xp[0][:, :, :, ds(i, 1)], zr[:, 0], OP.add)
                nc.vector.tensor_tensor(
                    z[:, 1], xp[1][:, :, :, ds(TB - 1 - i, 1)], zr[:, 1], OP.add)
                # gate order (i, f, o, g)
                nc.scalar.activation(z[:, :, 0:3, :], z[:, :, 0:3, :], AF.Sigmoid)
                nc.scalar.activation(z[:, :, 3, :], z[:, :, 3, :], AF.Tanh)
                ig = wpool.tile([H, 2, B], f32, tag="ig")
                nc.vector.tensor_tensor(ig[:], z[:, :, 0, :], z[:, :, 3, :],
                                        OP.mult)
                nc.vector.tensor_tensor(c_st[:], z[:, :, 1, :], c_st[:], OP.mult)
                nc.vector.tensor_tensor(c_st[:], c_st[:], ig[:], OP.add)
                th = wpool.tile([H, 2, B], f32, tag="th")
                nc.scalar.activation(th[:], c_st[:], AF.Tanh)
                nc.vector.tensor_tensor(
                    h_sf[:, :, ds(base_f + i + 1, 1)], z[:, 0, 2, :], th[:, 0],
                    OP.mult)
                nc.vector.tensor_tensor(
                    h_sb[:, :, ds(base_b - i, 1)], z[:, 1, 2, :], th[:, 1],
                    OP.mult)

        # ---- span gathers: gf/gb [H, (kind2, c8, s33)] ----
        par = cload(par_d, [1, 528], u8)
        parb = spool.tile([H, 528], u8)
        nc.gpsimd.partition_broadcast(parb[:], par[:], channels=H)
        gfr = spool.tile([H, 528, 2], f16)
        gbr = spool.tile([H, 528, 2], f16)
        nc.gpsimd.ap_gather(gfr[:], h_sf[:], gidx[:], channels=H,
                            num_elems=B * (L + 2) // 2, d=2, num_idxs=528)
        nc.gpsimd.ap_gather(gbr[:], h_sb[:], gidx[:], channels=H,
                            num_elems=B * (L + 2) // 2, d=2, num_idxs=528)
        gf = spool.tile([H, 528], f16)
        gb = spool.tile([H, 528], f16)
        nc.vector.select(gf[:], parb[:], gfr[:, :, 1], gfr[:, :, 0])
        nc.vector.select(gb[:], parb[:], gbr[:, :, 1], gbr[:, :, 0])
        nc.vector.tensor_tensor(gf[:], gf[:], amb[:], OP.mult)
        nc.vector.tensor_tensor(gb[:], gb[:], amb[:], OP.mult)
        gf3 = gf[:].rearrange("p (k c s) -> p k c s", k=2, c=B)
        gb3 = gb[:].rearrange("p (k c s) -> p k c s", k=2, c=B)

        def dbg(nm, ap):
            if dbg_d:
                nc.sync.dma_start(dbg_d[nm], ap)

        if dbg_d:
            dbg("gfs", gf[:])
            dbg("gbs", gb[:])

        # ---- per-comment heads ----
        for c in range(B):
            # span-rep K-chunks (80 rows each): fj-fi1 | bi-bj1 | fi1 | bj1
            dT = wpool.tile([H, 2, NS], f16, tag="dT")
            nc.vector.tensor_tensor(dT[:, 0], gf3[:, 1, c], gf3[:, 0, c],
                                    OP.subtract)
            nc.vector.tensor_tensor(dT[:, 1], gb3[:, 0, c], gb3[:, 1, c],
                                    OP.subtract)
            spch = [dT[:, 0], dT[:, 1], gf3[:, 0, c], gb3[:, 1, c]]

            # el/er for all (m,k,lr): [NS, 24]
            pel = p1pool.tile([NS, 24], f32, tag="ha")
            for b4 in range(4):
                nc.tensor.matmul(pel[:], spch[b4], va[:, b4, :],
                                 start=(b4 == 0), stop=(b4 == 3))
            el = wpool.tile([NS, 4, 3, 2], f16, tag="el")
            nc.scalar.copy(el[:], pel[:])

            # hp: [NS, (mk12, d256)] f16, two mk at a time
            if dbg_d and c == 0:
                dbg("el", el[:])
            hp = wpool.tile([NS, 12 * D], f16, tag="hp")
            for mk2 in range(6):
                php = p1pool.tile([NS, 2 * D], f32, tag="hb")
                for b4 in range(4):
                    nc.tensor.matmul(php[:], spch[b4],
                                     wgat[:, b4, ts(mk2, 2 * D)],
                                     start=(b4 == 0), stop=(b4 == 3))
                nc.scalar.copy(hp[:, ts(mk2, 2 * D)], php[:])

            # edge logits e[E, (m,k)] = lrelu(el[src] + er[dst]); ex = exp(e)*mask
            if dbg_d and c == 0:
                dbg("hp", hp[:])
            pes = p1pool.tile([E, 12], f32, tag="ha")
            for st in range(2):
                nc.tensor.matmul(pes[:, ts(st, 6)], gsrcT[:, c, st, :],
                                 el[0:32, 2 * st:2 * st + 2, :, 0:1],
                                 start=True, stop=False)
                nc.tensor.matmul(pes[:, ts(st, 6)], gdstT[:, c, st, :],
                                 el[0:32, 2 * st:2 * st + 2, :, 1:2],
                                 start=False, stop=True)
            e_sb = wpool.tile([E, 12], f32, tag="e_sb")
            nc.scalar.copy(e_sb[:], pes[:])
            nc.vector.scalar_tensor_tensor(e_sb[:], pes[:], 0.2, e_sb[:],
                                           OP.mult, OP.max)
            ex = wpool.tile([E, 12], f16, tag="ex")
            nc.scalar.activation(ex[:], e_sb[:], AF.Exp)
            nc.vector.tensor_tensor(ex[:], ex[:], m12[:, c, :], OP.mult)

            # den[A, 12] (clamped) and its reciprocal gathered back to edges
            if dbg_d and c == 0:
                dbg("e_sb", e_sb[:])
                dbg("ex", ex[:])
            pden = p1pool.tile([A, 12], f32, tag="hb")
            for st in range(2):
                nc.tensor.matmul(pden[:, ts(st, 6)], gdstE[0:E, c, st, :],
                                 ex[:, ts(st, 6)], start=True, stop=True)
            rden = wpool.tile([A, 12], f32, tag="rden")
            nc.vector.tensor_scalar_max(rden[:], pden[:], 1e-4)
            nc.vector.reciprocal(rden[:], rden[:])
            rdf = wpool.tile([A, 12], f16, tag="rdf")
            nc.vector.tensor_copy(rdf[:], rden[:])
            prd = p1pool.tile([E, 12], f32, tag="ha")
            for st in range(2):
                nc.tensor.matmul(prd[:, ts(st, 6)], gdstT[:, c, st, :],
                                 rdf[:, ts(st, 6)], start=True, stop=True)
            alpha = wpool.tile([E, 12], f32, tag="alpha")
            nc.vector.tensor_tensor(alpha[:], ex[:], prd[:], OP.mult)

            if dbg_d and c == 0:
                dbg("alpha", alpha[:])
            # GAT aggregate, transposed, with ELU'+1; zcT [128, (k,half), m, a]
            zcT = wpool.tile([128, 6, 4, A], f16, tag="zcT")
            for mk in range(12):
                m, kh = mk // 3, mk % 3
                st = m // 2
                psrc = p1pool.tile([E, D], f32, tag="hb")
                nc.tensor.matmul(psrc[:], gsrcT[:, c, st, :],
                                 hp[0:32, ts(mk, D)], start=True, stop=True)
                # weighted sources + GAT bias in row 48 (scatter adds it via
                # the ones row in gdstE)
                wsr = wpool.tile([E + 1, D], f16, tag="wsr")
                nc.sync.dma_start(wsr[48:49, :], bgat_d[:, ts(mk, D)])
                nc.vector.tensor_scalar_mul(wsr[0:E, :], psrc[:],
                                            alpha[:, mk:mk + 1])
                pz = p1pool.tile([128, 2, A], f32, tag="hb")
                for half in range(2):
                    nc.tensor.matmul(pz[:, half], wsr[:, ts(half, 128)],
                                     gdstE[:, c, st, :], start=True, stop=True)
                tmin = wpool.tile([128, 2, A], f32, tag="tmin")
                nc.vector.tensor_scalar_min(tmin[:], pz[:], 0.0)
                nc.scalar.activation(tmin[:], tmin[:], AF.Exp)
                nc.vector.scalar_tensor_tensor(
                    zcT[:, 2 * kh:2 * kh + 2, m, :], pz[:], 0.0, tmin[:],
                    OP.max, OP.add)

            if dbg_d and c == 0:
                dbg("zcT", zcT[:])
                dbg("wsr0", wsrcs[:, 0, :])
            # semantic attention scores
            wall = wpool.tile([A, 4], f16, tag="wall")
            sink = wpool.tile([A, DSEM], f16, tag="sink")
            for m in range(4):
                ps1 = p1pool.tile([A, DSEM], f32, tag="hb")
                for j in range(6):
                    nc.tensor.matmul(ps1[:], zcT[:, j, m, :], wsem[:, j, :],
                                     start=(j == 0), stop=False)
                nc.tensor.matmul(ps1[:], ones_row[:, 0:A], bsem[:],
                                 start=False, stop=True)
                tsb = wpool.tile([A, DSEM], f16, tag="tsb")
                nc.scalar.activation(tsb[:], ps1[:], AF.Tanh)
                nc.vector.scalar_tensor_tensor(
                    sink[:], tsb[:], 0.0, qbc[:], OP.bypass, OP.mult,
                    accum_out=wall[:, m:m + 1])
            pwm = p1pool.tile([1, 4], f32, tag="ha")
            nc.tensor.matmul(pwm[:], amcol[:, c:c + 1], wall[:],
                             start=True, stop=True)
            ex4 = wpool.tile([1, 4], f32, tag="ex4")
            nc.scalar.activation(ex4[:], pwm[:], AF.Exp,
                                 scale=invc[:, c:c + 1])
            sm4 = wpool.tile([1, 2], f32, tag="sm4")
            nc.vector.tensor_reduce(sm4[:, 0:1], ex4[:], mybir.AxisListType.X,
                                    OP.add)
            nc.vector.reciprocal(sm4[:, 1:2], sm4[:, 0:1])
            beta = wpool.tile([1, 4], f32, tag="beta")
            nc.vector.tensor_scalar_mul(beta[:], ex4[:], sm4[:, 1:2])
            if dbg_d and c == 0:
                dbg("wall", wall[:])
                dbg("beta", beta[:])
            bbc = wpool.tile([128, 4], f32, tag="bbc")
            nc.gpsimd.partition_broadcast(bbc[:], beta[:], channels=128)

            # zfinT = sum_m beta_m * zcT_m ; adu embeds
            zfinT = wpool.tile([128, 6, A], f16, tag="zfinT")
            for j in range(6):
                nc.vector.tensor_scalar_mul(zfinT[:, j, :], zcT[:, j, 0, :],
                                            bbc[:, 0:1])
                for m in range(1, 4):
                    nc.vector.scalar_tensor_tensor(
                        zfinT[:, j, :], zcT[:, j, m, :], bbc[:, m:m + 1],
                        zfinT[:, j, :], OP.mult, OP.add)
            pae = p1pool.tile([A, D], f32, tag="hb")
            for j in range(6):
                nc.tensor.matmul(pae[:], zfinT[:, j, :], wpred[:, j, :],
                                 start=(j == 0), stop=False)
            nc.tensor.matmul(pae[:], ones_row[:, 0:A], bpred[:],
                             start=False, stop=True)
            ae = wpool.tile([A, D], f16, tag="ae")
            nc.scalar.copy(ae[:], pae[:])
            if dbg_d and c == 0:
                dbg("ae", ae[:])
            aeT = wpool.tile([128, 2, A], f16, tag="aeT")
            for j in range(2):
                ptr = p1pool.tile([128, A], f16, tag="ha")
                nc.tensor.transpose(ptr[:], ae[:, ts(j, 128)], ident[0:A, 0:A])
                nc.scalar.copy(aeT[:, j, :], ptr[:])

            # cemb contribution to all three pool MLPs: r = cemb @ W*1[:320]
            pr1 = p1pool.tile([1, 48], f32, tag="hb")
            for b4 in range(4):
                nc.tensor.matmul(pr1[:], spch[b4][:, 32:33], w1a[:, b4, :],
                                 start=(b4 == 0), stop=(b4 == 3))
            rb = wpool.tile([1, 3, 16], f16, tag="rb")
            nc.vector.tensor_tensor(rb[:], pr1[:], b1[:], OP.add)

            out_row = wpool.tile([1, DWO], f32, tag="out_row")
            nc.vector.memset(out_row[:, 1284:1288], 0)

            # --- ADU attention pool ---
            ps1a = p1pool.tile([A, 16], f32, tag="ha")
            for j in range(2):
                nc.tensor.matmul(ps1a[:], aeT[:, j, :], w1adu[:, j, :],
                                 start=(j == 0), stop=False)
            nc.tensor.matmul(ps1a[:], ones_row[:, 0:A], rb[:, 0, :],
                             start=False, stop=True)
            scol = wpool.tile([A, 1], f32, tag="scol")
            sink16 = wpool.tile([A, 16], f16, tag="sink16")
            nc.vector.scalar_tensor_tensor(sink16[:], ps1a[:], 0.0,
                                           w2bc[0:A, 0:16], OP.max, OP.mult,
                                           accum_out=scol[:])
            if dbg_d and c == 0:
                dbg("scol", scol[:])
                dbg("rb", rb[:])
            exm = wpool.tile([A, 1], f16, tag="exm")
            nc.scalar.activation(exm[:], scol[:], AF.Exp)
            nc.vector.tensor_tensor(exm[:], exm[:], mal[:, c:c + 1], OP.mult)
            pd1 = p1pool.tile([1, 1], f32, tag="hb")
            nc.tensor.matmul(pd1[:], exm[:], ones_col[0:A, :],
                             start=True, stop=True)
            rd1 = wpool.tile([1, 1], f32, tag="rd1")
            nc.vector.tensor_scalar_max(rd1[:], pd1[:], 1e-30)
            nc.vector.reciprocal(rd1[:], rd1[:])
            ppl = p1pool.tile([1, D], f32, tag="ha")
            nc.tensor.matmul(ppl[:], exm[:], ae[:], start=True, stop=True)
            nc.vector.tensor_scalar_mul(out_row[:, 0:256], ppl[:], rd1[:])

            # --- pair attention pools (inner st=0, inter st=1) ---
            for st in range(2):
                base = 256 + 514 * st
                pe = wpool.tile([E, 514], f16, tag="pe")
                pse = p1pool.tile([E, D], f32, tag="hb")
                nc.tensor.matmul(pse[:], gsrcT[:, c, st, :], ae[:],
                                 start=True, stop=True)
                nc.scalar.copy(pe[:, 0:256], pse[:])
                psd = p1pool.tile([E, D], f32, tag="ha")
                nc.tensor.matmul(psd[:], gdstT[:, c, st, :], ae[:],
                                 start=True, stop=True)
                nc.scalar.copy(pe[:, 256:512], psd[:])
                nc.scalar.copy(pe[:, 512:514], rel2[:, c, st, :])
                if dbg_d and c == 0 and st == 0:
                    dbg("pe0", pe[:])
                peT = wpool.tile([128, 4, E], f16, tag="peT")
                for j in range(4):
                    gT = gsrcT if j < 2 else gdstT
                    ptp = p1pool.tile([128, E], f32, tag="hb")
                    nc.tensor.matmul(ptp[:], ae[:, ts(j % 2, 128)],
                                     gT[:, c, st, :], start=True, stop=True)
                    nc.scalar.copy(peT[:, j, :], ptp[:])
                ps1p = p1pool.tile([E, 16], f32, tag="ha")
                for j in range(4):
                    nc.tensor.matmul(ps1p[:], peT[:, j, :], w1pe[:, st, j, :],
                                     start=(j == 0), stop=False)
                nc.tensor.matmul(ps1p[:], relT[:, c, st, :], w1rel[:, st, :],
                                 start=False, stop=False)
                nc.tensor.matmul(ps1p[:], ones_row[:, 0:E], rb[:, 1 + st, :],
                                 start=False, stop=True)
                scol2 = wpool.tile([E, 1], f32, tag="scol2")
                sink2 = wpool.tile([E, 16], f16, tag="sink2")
                nc.vector.scalar_tensor_tensor(
                    sink2[:], ps1p[:], 0.0, w2bc[0:E, ts(1 + st, 16)],
                    OP.max, OP.mult, accum_out=scol2[:])
                if dbg_d and c == 0:
                    nc.sync.dma_start(dbg_d["scol2"][:, st:st + 1], scol2[:])
                exm2 = wpool.tile([E, 1], f16, tag="exm2")
                nc.scalar.activation(exm2[:], scol2[:], AF.Exp)
                nc.vector.tensor_tensor(exm2[:], exm2[:],
                                        mpair[:, st * B + c:st * B + c + 1],
                                        OP.mult)
                pd2 = p1pool.tile([1, 1], f32, tag="hb")
                nc.tensor.matmul(pd2[:], exm2[:], ones_col[0:E, :],
                                 start=True, stop=True)
                rd2 = wpool.tile([1, 1], f32, tag="rd2")
                nc.vector.tensor_scalar_max(rd2[:], pd2[:], 1e-30)
                nc.vector.reciprocal(rd2[:], rd2[:])
                pp1 = p1pool.tile([1, 512], f32, tag="ha")
                nc.tensor.matmul(pp1[:], exm2[:], pe[:, 0:512],
                                 start=True, stop=True)
                pp2 = p1pool.tile([1, 2], f32, tag="hb")
                nc.tensor.matmul(pp2[:], exm2[:], pe[:, 512:514],
                                 start=True, stop=True)
                nc.vector.tensor_scalar_mul(out_row[:, base:base + 512],
                                            pp1[:], rd2[:])
                nc.vector.tensor_scalar_mul(out_row[:, base + 512:base + 514],
                                            pp2[:], rd2[:])

            # cemb -> out_row[1288:1608]
            pcb = p1pool.tile([1, 4, H], f32, tag="ha")
            for b4 in range(4):
                nc.tensor.matmul(pcb[:, b4, :], spch[b4][:, 32:33], ident[:],
                                 start=True, stop=True)
            nc.scalar.copy(out_row[:, 1288:1608], pcb[:])

            nc.sync.dma_start(wo_d[c:c + 1, :], out_row[:])

    nc.compile()
    return nc


# ---------------------------------------------------------------------------
# host-side packing
# ---------------------------------------------------------------------------

def _wrap16(flat, channels):
    """Wrap a flat index list into ap_gather's [channels, n//16] layout."""
    n = len(flat)
    out = np.zeros((channels, n // 16), np.int16)
    j = np.arange(n)
    for g in range(channels // 16):
        out[g * 16 + j % 16, j // 16] = flat
    return out


def pack_weights(inp):
    """Pack shared weights (identical for every core)."""
    f16 = np.float16
    perm = np.r_[0:80, 80:160, 240:320, 160:240]     # torch ifgo -> (i,f,o,g)

    w = {}
    tscale = float(np.abs(inp['token_embed']).max()) / 127.0
    w['_tscale'] = tscale
    wih2 = np.stack([inp['Wih_f'][perm], inp['Wih_b'][perm]]) * tscale
    # lhsT chunks [128, (dir,gate), k, 80]: wih[p, dg, k, m] = wih2[d, g*80+m, k*128+p]
    w['wih'] = np.ascontiguousarray(
        wih2.reshape(2, 4, H, 6, 128).transpose(4, 0, 1, 3, 2)
    ).reshape(128, 8, 6, H).astype(f16)
    whh2 = np.stack([inp['Whh_f'][perm], inp['Whh_b'][perm]])   # [2,320,80]
    w['whh'] = np.ascontiguousarray(
        whh2.reshape(2, 4, H, H).transpose(3, 0, 1, 2)
    ).reshape(H, 8, H).astype(f16)
    bg2 = np.stack([inp['b_f'][perm], inp['b_b'][perm]])        # [2,320]
    w['bg'] = np.ascontiguousarray(
        bg2.reshape(2, 4, H).transpose(2, 0, 1)).reshape(H, 8).astype(np.float32)

    wg = inp['W_gat'].astype(np.float32)                        # [4,3,320,256]
    va = np.einsum('mkdo,mko->mkd', wg, inp['a_l'])             # [4,3,320]
    vb = np.einsum('mkdo,mko->mkd', wg, inp['a_r'])
    vab = np.stack([va, vb], -1).reshape(4, 3, 4, H, 2)         # m,k,blk,80,lr
    w['va'] = np.ascontiguousarray(
        vab.transpose(3, 2, 0, 1, 4)).reshape(H, 4, 24).astype(f16)
    # wgat rhs chunks [80, blk, (mk, 256)]
    w['wgat'] = np.ascontiguousarray(
        wg.reshape(4, 3, 4, H, D).transpose(3, 2, 0, 1, 4)
    ).reshape(H, 4, 12 * D).astype(f16)
    w['bgat'] = inp['b_gat'].reshape(1, 12 * D).astype(f16)

    ws = inp['W_sem'].astype(np.float32)                        # [768,128]
    w['wsem'] = np.ascontiguousarray(
        ws.reshape(6, 128, DSEM).transpose(1, 0, 2)).astype(f16)
    w['bsem'] = (inp['b_sem'] - ws.sum(0)).reshape(1, DSEM).astype(f16)
    w['qsem'] = inp['q_sem'].reshape(1, DSEM).astype(f16)
    wp = inp['W_pred'].astype(np.float32)                       # [768,256]
    w['wpred'] = np.ascontiguousarray(
        wp.reshape(6, 128, D).transpose(1, 0, 2)).astype(f16)
    w['bpred'] = (inp['b_pred'] - wp.sum(0)).reshape(1, D).astype(f16)

    w1s = [inp['W_adu1'].astype(np.float32), inp['W_inn1'].astype(np.float32),
           inp['W_int1'].astype(np.float32)]
    w1a = np.stack([wm[:SPAN].reshape(4, H, 16) for wm in w1s], 2)  # blk,80,pool,16
    w['w1a'] = np.ascontiguousarray(w1a.transpose(1, 0, 2, 3)).reshape(
        H, 4, 48).astype(f16)
    w['w1adu'] = np.ascontiguousarray(
        w1s[0][SPAN:SPAN + D].reshape(2, 128, 16).transpose(1, 0, 2)
    ).astype(f16)
    w1pe = np.stack([w1s[1][SPAN:SPAN + 512], w1s[2][SPAN:SPAN + 512]])
    w['w1pe'] = np.ascontiguousarray(
        w1pe.reshape(2, 4, 128, 16).transpose(2, 0, 1, 3)).astype(f16)
    w['w1rel'] = np.stack([w1s[1][832:834], w1s[2][832:834]]).transpose(
        1, 0, 2).astype(f16)
    w['b1'] = np.stack([inp['b_adu1'], inp['b_inn1'], inp['b_int1']]).reshape(
        1, 3, 16).astype(f16)
    w['w2'] = np.stack([inp['W_adu2'][:, 0], inp['W_inn2'][:, 0],
                        inp['W_int2'][:, 0]]).reshape(1, 48).astype(np.float32)
    w['ident'] = np.eye(H, dtype=f16)
    return w


def pack_core(inp, core, wshared):
    """Per-core inputs (tokens, graph one-hots, masks, gather indices)."""
    f16 = np.float16
    sl = slice(core * B, (core + 1) * B)
    m = {k: v for k, v in wshared.items() if not k.startswith('_')}

    ts_ = wshared['_tscale']
    tok = np.clip(np.round(inp['token_embed'][sl].astype(np.float32) / ts_),
                  -127, 127).astype(np.int8)                    # [8,1024,768]
    m['tok'] = np.ascontiguousarray(
        tok.reshape(B, NBLK, TB, 6, 128).transpose(3, 4, 1, 0, 2)
    ).reshape(6, 128, B * L)

    cs = inp['comment_spans'][sl].astype(np.int64)              # [8,2]
    asp = inp['adu_spans'][sl].astype(np.int64)                 # [8,32,2]
    i_all = np.concatenate([asp[:, :, 0], cs[:, None, 0]], 1)   # [8,33]
    j_all = np.concatenate([asp[:, :, 1], cs[:, None, 1]], 1)
    coff = np.arange(B)[:, None] * (L + 2)
    flat = np.concatenate([(coff + i_all).ravel(),
                           (coff + j_all + 1).ravel()]).astype(np.int64)
    m['gidx'] = _wrap16((flat >> 1).astype(np.int16), H)
    m['par'] = (flat & 1).reshape(1, 528).astype(np.uint8)

    am = inp['adu_masks'][sl].astype(np.float32)                # [8,32]
    am_ext = np.concatenate([am, np.ones((B, 1), np.float32)], 1)  # [8,33]
    m['amrow'] = np.tile(am_ext.ravel(), 2).reshape(1, 2 * B * NS).astype(f16)
    m['amcol'] = am.T.astype(f16)
    m['invc'] = (1.0 / np.maximum(am.sum(1), 1.0)).reshape(1, B).astype(
        np.float32)
    m['mal'] = (am * inp['local_masks'][sl]).T.astype(f16)

    im = inp['inner_mask'][sl].astype(np.float32)               # [8,48]
    tm = inp['inter_mask'][sl].astype(np.float32)
    m['mpair'] = np.concatenate([im, tm], 0).T.astype(f16)      # [48, 2*8]
    irel = inp['inner_rel'][sl].astype(np.int64)
    trel = inp['inter_rel'][sl].astype(np.int64)
    m4 = np.stack([im * (irel == 0), im * (irel == 1),
                   tm * (trel == 0), tm * (trel == 1)], 1)      # [8,4,48]
    m['m12'] = np.ascontiguousarray(
        np.repeat(m4[:, :, None, :], 3, 2).transpose(3, 0, 1, 2)
    ).reshape(E, B, 12).astype(f16)

    eye = np.eye(A, dtype=np.float32)
    src = np.stack([inp['inner_src'][sl], inp['inter_dst'][sl]], 1)  # [8,2,48]
    dst = np.stack([inp['inner_dst'][sl], inp['inter_src'][sl]], 1)
    gsrc = eye[src.astype(np.int64)]                            # [8,2,48,32]
    gdst = eye[dst.astype(np.int64)]
    m['gsrcT'] = np.ascontiguousarray(
        gsrc.transpose(3, 0, 1, 2)).astype(f16)                 # [32,8,2,48]
    m['gdstT'] = np.ascontiguousarray(gdst.transpose(3, 0, 1, 2)).astype(f16)
    gde = np.concatenate([gdst, np.ones((B, 2, 1, A), np.float32)], 2)
    m['gdstE'] = np.ascontiguousarray(
        gde.transpose(2, 0, 1, 3)).astype(f16)                  # [49,8,2,32]
    rel = np.stack([irel, trel], 1).astype(np.float32)          # [8,2,48]
    r2 = np.stack([rel, 1.0 - rel], -1)                         # [8,2,48,2]
    m['relT'] = np.ascontiguousarray(
        r2.transpose(3, 0, 1, 2)).astype(f16)                   # [2,8,2,48]
    m['rel2'] = np.ascontiguousarray(
        r2.transpose(2, 0, 1, 3)).astype(f16)                   # [48,8,2,2]
    return m


# ---------------------------------------------------------------------------
# host-side epilogue (comment-compressor LSTM)
# ---------------------------------------------------------------------------

def compressor(wo, inp):
    Wt = inp['Whh_c'].T.astype(np.float32)
    xp = wo @ inp['Wih_c'].T.astype(np.float32) + inp['b_c']    # [64, 800]
    h = np.zeros(CTX, np.float32)
    c = np.zeros(CTX, np.float32)
    hs = np.empty((C, CTX), np.float32)

    def sig(v):
        return 1.0 / (1.0 + np.exp(-v))

    for t in range(C):
        z = xp[t] + h @ Wt
        i, f, g, o = (z[:CTX], z[CTX:2 * CTX], z[2 * CTX:3 * CTX], z[3 * CTX:])
        c = sig(f) * c + sig(i) * np.tanh(g)
        h = sig(o) * np.tanh(c)
        hs[t] = h
    return hs


# ---------------------------------------------------------------------------
# entry point
# ---------------------------------------------------------------------------

def kernel(**inputs):
    global _compiled
    inp = {k: np.asarray(v) for k, v in inputs.items()}

    wshared = pack_weights(inp)
    in_maps = [pack_core(inp, core, wshared) for core in range(NCORE)]

    if _compiled is None:
        _compiled = build_kernel()
    globals()['_last_in_maps'] = in_maps

    from concourse.bass_utils import run_bass_kernel_spmd
    import time as _time
    _t0 = _time.time()
    res = run_bass_kernel_spmd(_compiled, in_maps,
                               core_ids=list(range(NCORE)))
    globals()['_last_exec_ns'] = res.exec_time_ns
    globals()['_last_dispatch_s'] = _time.time() - _t0

    wo = np.concatenate([res.results[core]["wo"] for core in range(NCORE)], 0)
    wo[:, 1284:1288] = inp['info_scores'].astype(np.float32)

    ctx = compressor(wo, inp)
    return np.concatenate([ctx, wo], -1).astype(np.float32)
